# revision 8
# baseline (speedup 1.0000x reference)
"""HDMR kernel, arch T: arch G + tabulated first-order nets.

The 8 first-order sub-networks are univariate, so they are evaluated on
a 512-point grid (as 4 "virtual" stream nets, 2 singles x 512 grid cols
each) instead of on the batch: the sigmoid stream shrinks from 138 to
132 fused ops.  Tables are extracted with DVE w_out-scaling + GPSIMD
partition_all_reduce (output replicated across partitions, gather-
ready), then Catmull-Rom interpolated at the batch x values with GPSIMD
ap_gather + DVE arithmetic, using host-precomputed int16 indices and
f32 basis weights (pure functions of the input x).  The interpolated
contribution leaves as a second output that the host adds during
unshard.  Everything else matches arch G (fused 2048-wide sigmoids,
bias folding with compensation matmuls, skewed stream, DVE output
layer).
"""

import itertools
from contextlib import ExitStack

import numpy as np
import ml_dtypes

BF16 = ml_dtypes.bfloat16

NUM_VARS = 8
HID = 128
B = 8192
NCORES = 8
BC = B // NCORES
HALF = BC // 2

PAIRS = list(itertools.combinations(range(NUM_VARS), 2))
TRIPS = list(itertools.combinations(range(NUM_VARS), 3))
N1, N2, N3 = NUM_VARS, len(PAIRS), len(TRIPS)
NNETS = N1 + N2 + N3  # 92 original nets

NV = 4  # virtual tab nets, each = 2 singles x 512 grid cols
DEVV = (16, 34, 52, 70)  # scattered device slots for the virtual nets
NDEV = NV + (NNETS - N1)  # 88 stream nets
G = 512  # grid points per single
GLO, GHI = -4.9, 4.9
GSTEP = (GHI - GLO) / (G - 1)
GRID_W_ROW = 9  # xT row carrying the grid (per 32-row base block)
GRID_1_ROW = 10  # xT row carrying ones for the tab bias

WIN_K = 32
WIN_NPB = 3
WIN_BLOCKS = (NNETS + WIN_NPB - 1) // WIN_NPB  # 31 (orig layout kept)
ONES_ROW = 8
PE_OUT_FIRST = NDEV - 4  # device nets >= this use PE output matmuls

WI_CHUNKS = [(2, 5), (5, 9), (9, 16), (16, WIN_BLOCKS), (0, 2)]
WH_CHUNKS = [(0, 2), (2, 5), (5, 10)] + [
    (10 + 6 * i, min(16 + 6 * i, NNETS)) for i in range(14)
]

_CACHE = {}


def _is_v(n):
    return n in DEVV


def _dev_orig(n, half):
    """Original net id for (device net, half)."""
    if _is_v(n):
        return 2 * DEVV.index(n) + half
    return n - sum(1 for v in DEVV if v < n) + N1


def _coeffs():
    dim = NNETS + 1
    e = np.eye(dim, dtype=np.float64)
    f0v = e[NNETS]
    f1 = [e[j] - f0v for j in range(N1)]
    f2 = [e[N1 + p] - f1[a] - f1[b] - f0v for p, (a, b) in enumerate(PAIRS)]
    f3 = [
        e[N1 + N2 + t] - f2[i] - f2[j] - f2[k] - f1[i] - f1[j] - f1[k] - f0v
        for t, (i, j, k) in enumerate(TRIPS)
    ]
    final = f0v + sum(f1) + sum(f2) + sum(f3)
    return final[:NNETS], final[NNETS]


def _net_vars():
    return [(j,) for j in range(N1)] + PAIRS + TRIPS


def _stream():
    N = NDEV
    S = [("in", i) for i in range(6)] + [("h1", 0), ("h1", 1)]
    for m in range(N - 6):
        S.append(("in", m + 6))
        S.append(("h1", m + 2))
        S.append(("h2", m))
    S += [
        ("h1", N - 4), ("h2", N - 6), ("h1", N - 3), ("h2", N - 5),
        ("h1", N - 2), ("h1", N - 1),
        ("h2", N - 4), ("h2", N - 3), ("h2", N - 2), ("h2", N - 1),
    ]
    assert len(S) == 3 * N
    pos = {b: p for p, b in enumerate(S)}
    for n in range(N):
        assert pos[("h1", n)] // 2 - pos[("in", n)] // 2 >= 2, n
        assert pos[("h2", n)] // 2 - pos[("h1", n)] // 2 >= 2, n
    return S


def _plan():
    """opbias per fused op (orig net, layer) from the first REAL h-block;
    comp rows per (device net, layer, half) for h-block halves whose
    orig bias differs from the op bias (virtual halves always differ)."""
    S = _stream()
    fops = [(S[2 * j], S[2 * j + 1]) for j in range(len(S) // 2)]
    opbias = []  # per op: (orig net, layer) or None
    comp = []  # rows: (orig own net, layer, opbias)
    comp_idx = {}  # (dev net, layer, half) -> row
    in_adj = {}  # dev net -> opbias of its op
    for b0, b1 in fops:
        hb = None
        for kind, n in (b0, b1):
            if kind != "in" and not _is_v(n):
                hb = (kind, n)
                break
        ob = (_dev_orig(hb[1], 0), 0 if hb[0] == "h1" else 1) if hb else None
        opbias.append(ob)
        for kind, n in (b0, b1):
            if kind == "in":
                in_adj[n] = ob
                continue
            l = 0 if kind == "h1" else 1
            if _is_v(n):
                for half in (0, 1):
                    comp_idx[(n, l, half)] = len(comp)
                    comp.append((_dev_orig(n, half), l, ob))
            elif (_dev_orig(n, 0), l) != ob:
                row = len(comp)
                comp.append((_dev_orig(n, 0), l, ob))
                comp_idx[(n, l, 0)] = row
                comp_idx[(n, l, 1)] = row
    return S, fops, opbias, comp, comp_idx, in_adj


def _build_bass():
    from concourse import tile
    from concourse.bacc import Bacc
    import concourse.mybir as mybir
    from concourse.alu_op_type import AluOpType
    from concourse import bass_isa

    f32 = mybir.dt.float32
    f32r = mybir.dt.float32r
    bf16 = mybir.dt.bfloat16
    i16 = mybir.dt.int16
    SIG = mybir.ActivationFunctionType.Sigmoid
    IDENT = mybir.ActivationFunctionType.Identity

    nc = Bacc(
        "TRN2",
        target_bir_lowering=False,
        debug=False,
        enable_asserts=False,
        num_devices=1,
    )

    S, fops, opbias, comp, comp_idx, in_adj = _plan()

    xT_d = nc.dram_tensor("xT", [WIN_NPB * WIN_K, BC], f32r, kind="ExternalInput")
    w_in_d = nc.dram_tensor(
        "w_in", [WIN_NPB * WIN_K, WIN_BLOCKS * HID], f32r, kind="ExternalInput"
    )
    w_h_d = nc.dram_tensor("w_h", [HID, NNETS * 2 * HID], f32r, kind="ExternalInput")
    bh_d = nc.dram_tensor("bh_cols", [HID, 2 * NNETS], f32, kind="ExternalInput")
    cmp_d = nc.dram_tensor("cmp", [1, len(comp) * HID], bf16, kind="ExternalInput")
    w_out_d = nc.dram_tensor("w_out", [HID, NNETS], f32r, kind="ExternalInput")
    cb_d = nc.dram_tensor("cb", [1, 1], f32, kind="ExternalInput")
    ones512_d = nc.dram_tensor("ones512", [1, HALF], bf16, kind="ExternalInput")
    ones128_d = nc.dram_tensor("ones128", [HID, 1], f32, kind="ExternalInput")
    idx_d = nc.dram_tensor("idx", [HID, 4 * N1 * 8], i16, kind="ExternalInput")
    basis_d = nc.dram_tensor("basis", [HID, N1 * 3 * HID], f32, kind="ExternalInput")
    out_d = nc.dram_tensor("out", [1, BC], f32, kind="ExternalOutput")
    out2_d = nc.dram_tensor("out2", [HID, HID], f32, kind="ExternalOutput")

    with tile.TileContext(nc) as tc:
        with ExitStack() as ctx:
            const = ctx.enter_context(tc.tile_pool(name="const", bufs=1))
            psp = ctx.enter_context(tc.tile_pool(name="psp", bufs=2, space="PSUM"))
            hp = ctx.enter_context(tc.tile_pool(name="hp", bufs=7))
            tmpp = ctx.enter_context(tc.tile_pool(name="tmpp", bufs=1))
            gp = ctx.enter_context(tc.tile_pool(name="gp", bufs=6))
            trp = ctx.enter_context(tc.tile_pool(name="trp", bufs=1))

            warm = const.tile([1, 2], f32, tag="warm", name="warm_sb")
            nc.gpsimd.memset(warm[:, 0:1], 0.0)
            nc.scalar.activation(warm[:, 1:2], warm[:, 0:1], SIG)

            wi_tiles = []
            for ci, (lo, hi) in enumerate(WI_CHUNKS):
                t = const.tile(
                    [WIN_NPB * WIN_K, (hi - lo) * HID], f32r,
                    tag=f"wi{ci}", name=f"wi{ci}",
                )
                wi_tiles.append(t)
            wh_tiles = []
            for ci, (lo, hi) in enumerate(WH_CHUNKS):
                t = const.tile(
                    [HID, (hi - lo) * 2 * HID], f32r, tag=f"wh{ci}", name=f"wh{ci}"
                )
                wh_tiles.append(t)
            xT_sb = const.tile([WIN_NPB * WIN_K, BC], f32r, tag="xT", name="xT_sb")
            bh_sb = const.tile([HID, 2 * NNETS], f32, tag="bh", name="bh_sb")
            cmp_sb = const.tile([1, len(comp) * HID], bf16, tag="cmp", name="cmp_sb")
            w_out_sb = const.tile([HID, NNETS], f32r, tag="w_out", name="w_out_sb")
            cb_sb = const.tile([1, 1], f32, tag="cb", name="cb_sb")
            ones512 = const.tile([1, HALF], bf16, tag="ones512", name="ones512")
            ones128 = const.tile([HID, 1], f32, tag="ones128", name="ones128")
            idx_sb = const.tile([HID, 4 * N1 * 8], i16, tag="idx", name="idx_sb")
            bp = ctx.enter_context(tc.tile_pool(name="bp", bufs=2))
            basis_tiles = [
                bp.tile([HID, 2 * 3 * HID], f32, tag="B", name=f"B{v}")
                for v in range(NV)
            ]
            ysum = const.tile([HID, HID], f32, tag="ysum", name="ysum_sb")

            def wi_dma(ci):
                lo, hi = WI_CHUNKS[ci]
                nc.sync.dma_start(
                    wi_tiles[ci][:], w_in_d.ap()[:, lo * HID: hi * HID]
                )

            def wh_dma(ci):
                lo, hi = WH_CHUNKS[ci]
                nc.sync.dma_start(
                    wh_tiles[ci][:], w_h_d.ap()[:, lo * 2 * HID: hi * 2 * HID]
                )

            nc.gpsimd.dma_start(
                wi_tiles[0][:],
                w_in_d.ap()[:, WI_CHUNKS[0][0] * HID: WI_CHUNKS[0][1] * HID],
            )
            nc.scalar.dma_start(xT_sb[:, HALF:BC], xT_d.ap()[:, HALF:BC])
            nc.sync.dma_start(xT_sb[:, 0:HALF], xT_d.ap()[:, 0:HALF])
            nc.sync.dma_start(bh_sb[:], bh_d.ap())
            nc.sync.dma_start(idx_sb[:], idx_d.ap())
            nc.sync.dma_start(cmp_sb[:], cmp_d.ap())
            nc.sync.dma_start(ones512[:], ones512_d.ap())
            wh_dma(2)
            wi_dma(1)
            wh_dma(3)
            nc.sync.dma_start(w_out_sb[:], w_out_d.ap())
            wh_dma(4)
            wi_dma(2)
            wi_dma(3)
            wi_dma(4)
            nc.sync.dma_start(ones128[:], ones128_d.ap())
            for ci in range(5, 10):
                wh_dma(ci)
            wh_dma(0)
            wh_dma(1)
            for v in range(NV):
                nc.sync.dma_start(
                    basis_tiles[v][:],
                    basis_d.ap()[:, v * 2 * 3 * HID: (v + 1) * 2 * 3 * HID],
                )
            for ci in range(10, len(WH_CHUNKS)):
                wh_dma(ci)
            nc.sync.dma_start(cb_sb[:], cb_d.ap())

            acc = const.tile([HID, BC], f32, tag="acc0", name="acc0")
            acc_used = [False]

            def wi_lookup(cblk):
                for ci, (lo, hi) in enumerate(WI_CHUNKS):
                    if lo <= cblk < hi:
                        return wi_tiles[ci], lo
                raise AssertionError(cblk)

            def wh_lookup(o):
                for ci, (lo, hi) in enumerate(WH_CHUNKS):
                    if lo <= o < hi:
                        return wh_tiles[ci], lo
                raise AssertionError(o)

            def in_mms(n, P, half_b):
                for h in (0, 1):
                    o = _dev_orig(n, h)
                    jb, cblk = o % WIN_NPB, o // WIN_NPB
                    wt, lo = wi_lookup(cblk)
                    lhsT = wt[WIN_K * jb: WIN_K * jb + WIN_K,
                              (cblk - lo) * HID: (cblk - lo + 1) * HID]
                    rhs = xT_sb[WIN_K * jb: WIN_K * jb + WIN_K,
                                h * HALF: (h + 1) * HALF]
                    nc.tensor.matmul(
                        P[:, half_b * BC + h * HALF: half_b * BC + (h + 1) * HALF],
                        lhsT, rhs, start=True, stop=True,
                    )

            def h_mms(n, layer, P, half_b, src):
                Hs, sh = src
                for h in (0, 1):
                    o = _dev_orig(n, h)
                    wt, lo = wh_lookup(o)
                    col = ((o - lo) * 2 + layer) * HID
                    ci = comp_idx.get((n, layer, h))
                    out = P[:, half_b * BC + h * HALF: half_b * BC + (h + 1) * HALF]
                    nc.tensor.matmul(
                        out, wt[:, col: col + HID],
                        Hs[:, sh * BC + h * HALF: sh * BC + (h + 1) * HALF],
                        start=True, stop=ci is None,
                    )
                    if ci is not None:
                        nc.tensor.matmul(
                            out, cmp_sb[0:1, ci * HID: (ci + 1) * HID],
                            ones512[:], start=False, stop=True,
                        )

            def interp(s, tbl):
                """Catmull-Rom interp of table s at the batch x values:
                y = g1 + b0*(g0-g1) + b2*(g2-g1) + b3*(g3-g1)."""
                gt = []
                for k in range(4):
                    gk = gp.tile([HID, HID], f32, tag="G", name=f"G{s}_{k}")
                    nc.gpsimd.ap_gather(
                        gk[:], tbl[:],
                        idx_sb[:, (k * N1 + s) * 8: (k * N1 + s) * 8 + 8],
                        channels=HID, num_elems=G, d=1, num_idxs=HID,
                    )
                    gt.append(gk)

                bt = basis_tiles[s // 2]

                def bsl(slot):
                    return bt[:, ((s % 2) * 3 + slot) * HID:
                              ((s % 2) * 3 + slot + 1) * HID]

                b1t = gp.tile([HID, HID], f32, tag="G", name=f"b1_{s}")
                nc.vector.tensor_scalar(
                    b1t[:], bsl(0), -1.0, 1.0, AluOpType.mult, AluOpType.add
                )
                nc.vector.tensor_tensor(b1t[:], b1t[:], bsl(1), AluOpType.subtract)
                nc.vector.tensor_tensor(b1t[:], b1t[:], bsl(2), AluOpType.subtract)
                y = gp.tile([HID, HID], f32, tag="G", name=f"y{s}")
                nc.vector.tensor_tensor(y[:], gt[1][:], b1t[:], AluOpType.mult)
                for slot, k in ((0, 0), (1, 2), (2, 3)):
                    m = gp.tile([HID, HID], f32, tag="G", name=f"m{s}_{k}")
                    nc.vector.tensor_tensor(m[:], gt[k][:], bsl(slot), AluOpType.mult)
                    nc.vector.tensor_tensor(y[:], y[:], m[:], AluOpType.add)
                if s == 0:
                    nc.vector.tensor_copy(ysum[:], y[:])
                else:
                    nc.vector.tensor_tensor(ysum[:], ysum[:], y[:], AluOpType.add)

            loc = {}
            for j, blocks in enumerate(fops):
                P = psp.tile([HID, 2 * BC], f32, tag="P", name=f"P{j}")
                for half_b, (kind, n) in enumerate(blocks):
                    if kind == "in":
                        in_mms(n, P, half_b)
                    elif kind == "h1":
                        h_mms(n, 0, P, half_b, loc[("in", n)])
                    else:
                        h_mms(n, 1, P, half_b, loc[("h1", n)])
                H = hp.tile([HID, 2 * BC], f32r, tag="H", name=f"H{j}")
                ob = opbias[j]
                bias = (
                    bh_sb[:, ob[0] * 2 + ob[1]: ob[0] * 2 + ob[1] + 1]
                    if ob is not None else 0.0
                )
                nc.scalar.activation(H[:], P[:], SIG, bias=bias)
                for half_b, (kind, n) in enumerate(blocks):
                    loc[(kind, n)] = (H, half_b)
                    if kind != "h2":
                        continue
                    if _is_v(n):
                        for h in (0, 1):
                            s = 2 * DEVV.index(n) + h
                            tmp = tmpp.tile([HID, G], f32, tag="tmp",
                                            name=f"tmp{s}")
                            nc.vector.tensor_scalar(
                                tmp[:],
                                H[:, half_b * BC + h * HALF:
                                  half_b * BC + (h + 1) * HALF],
                                w_out_sb[:, s: s + 1].bitcast(f32),
                                None, AluOpType.mult,
                            )
                            tbl = trp.tile([HID, G], f32, tag="T",
                                           name=f"T{s}")
                            nc.gpsimd.partition_all_reduce(
                                tbl[:], tmp[:], HID, bass_isa.ReduceOp.add,
                            )
                            interp(s, tbl)
                        if n == DEVV[-1]:
                            nc.sync.dma_start(out2_d.ap(), ysum[:])
                    elif n < PE_OUT_FIRST:
                        o = _dev_orig(n, 0)
                        h2s = H[:, half_b * BC: (half_b + 1) * BC]
                        w_col = w_out_sb[:, o: o + 1].bitcast(f32)
                        if not acc_used[0]:
                            acc_used[0] = True
                            nc.vector.tensor_scalar(
                                acc[:], h2s, w_col, None, AluOpType.mult
                            )
                        else:
                            nc.vector.scalar_tensor_tensor(
                                acc[:], h2s, w_col, acc[:],
                                AluOpType.mult, AluOpType.add,
                            )

            Pf = psp.tile([HID, 2 * BC], f32, tag="P", name="Pfin")
            for h in (0, 1):
                nc.tensor.matmul(
                    Pf[0:1, h * HALF: (h + 1) * HALF],
                    ones128[:], acc[:, h * HALF: (h + 1) * HALF],
                    start=True, stop=False,
                )
            for h in (0, 1):
                for n in range(PE_OUT_FIRST, NDEV):
                    Hs, sh = loc[("h2", n)]
                    o = _dev_orig(n, 0)
                    nc.tensor.matmul(
                        Pf[0:1, h * HALF: (h + 1) * HALF],
                        w_out_sb[:, o: o + 1],
                        Hs[:, sh * BC + h * HALF: sh * BC + (h + 1) * HALF],
                        start=False, stop=n == NDEV - 1,
                    )
            out_sb = const.tile([1, BC], f32, tag="out_sb", name="out_sb")
            nc.scalar.activation(
                out_sb[:, 0:HALF], Pf[0:1, 0:HALF], IDENT, bias=cb_sb[:]
            )
            nc.vector.tensor_scalar(
                out_sb[:, HALF:BC], Pf[0:1, HALF:BC], cb_sb[:], None,
                AluOpType.add,
            )
            nc.sync.dma_start(out_d.ap()[:, 0:HALF], out_sb[:, 0:HALF])
            nc.sync.dma_start(out_d.ap()[:, HALF:BC], out_sb[:, HALF:BC])

    nc.finalize()
    return nc


def _prep_weights(inputs):
    c, c_f0 = _coeffs()
    nets = _net_vars()
    _, _, _, comp, _, in_adj = _plan()

    groups = []
    for tag, count in (("1", N1), ("2", N2), ("3", N3)):
        groups.append(
            dict(
                W_in=np.asarray(inputs[f"W_in_{tag}"], np.float32),
                b_in=np.asarray(inputs[f"b_in_{tag}"], np.float32),
                W_h=np.asarray(inputs[f"W_h_{tag}"], np.float32),
                b_h=np.asarray(inputs[f"b_h_{tag}"], np.float32),
                W_out=np.asarray(inputs[f"W_out_{tag}"], np.float32),
                b_out=np.asarray(inputs[f"b_out_{tag}"], np.float32),
                n=count,
            )
        )

    w_in = np.zeros((WIN_NPB * WIN_K, WIN_BLOCKS * HID), np.float32)
    w_h = np.zeros((HID, NNETS * 2 * HID), np.float32)
    bh_cols = np.zeros((HID, 2 * NNETS), np.float32)
    w_out = np.zeros((HID, NNETS), np.float32)
    b_in_all = np.zeros((NNETS, HID), np.float32)
    cb = np.float64(c_f0) * np.float64(inputs["f0"])

    n = 0
    for g in groups:
        for k in range(g["n"]):
            vars_n = nets[n]
            cblk, j = divmod(n, WIN_NPB)
            if n >= N1:  # singles' batch slots stay zero (tab rows instead)
                for i, v in enumerate(vars_n):
                    w_in[j * WIN_K + v, cblk * HID: (cblk + 1) * HID] = (
                        g["W_in"][k, :, i]
                    )
            b_in_all[n] = g["b_in"][k]
            for l in range(2):
                w_h[:, (n * 2 + l) * HID: (n * 2 + l + 1) * HID] = g["W_h"][k, l].T
                bh_cols[:, n * 2 + l] = g["b_h"][k, l]
            w_out[:, n] = c[n] * g["W_out"][k, 0, :]
            cb += np.float64(c[n]) * np.float64(g["b_out"][k])
            n += 1
    assert n == NNETS

    def ob_vec(ob):
        return bh_cols[:, ob[0] * 2 + ob[1]] if ob is not None else 0.0

    for dn, ob in in_adj.items():
        if _is_v(dn):
            for h in (0, 1):
                s = 2 * DEVV.index(dn) + h
                cblk, jb = divmod(s, WIN_NPB)
                w_in[jb * WIN_K + GRID_W_ROW, cblk * HID: (cblk + 1) * HID] = (
                    groups[0]["W_in"][s, :, 0]
                )
                w_in[jb * WIN_K + GRID_1_ROW, cblk * HID: (cblk + 1) * HID] = (
                    b_in_all[s] - ob_vec(ob)
                )
        else:
            o = dn - NV + N1
            cblk, jb = divmod(o, WIN_NPB)
            w_in[jb * WIN_K + ONES_ROW, cblk * HID: (cblk + 1) * HID] = (
                b_in_all[o] - ob_vec(ob)
            )

    cmp = np.zeros((1, len(comp) * HID), np.float32)
    for ci, (o, l, ob) in enumerate(comp):
        cmp[0, ci * HID: (ci + 1) * HID] = bh_cols[:, o * 2 + l] - ob_vec(ob)

    return dict(
        w_in=w_in,
        w_h=w_h,
        bh_cols=bh_cols,
        cmp=cmp.astype(BF16),
        w_out=w_out,
        cb=np.array([[cb]], np.float32),
        ones512=np.ones((1, HALF), BF16),
        ones128=np.ones((HID, 1), np.float32),
    )


def make_in_maps(inputs):
    w = _prep_weights(inputs)
    x = np.asarray(inputs["x"], np.float32)
    assert np.abs(x).max() < GHI - 2 * GSTEP, "x outside tab grid"
    grid = (GLO + GSTEP * np.arange(G)).astype(np.float32)

    xT = np.zeros((WIN_NPB * WIN_K, B), np.float32)
    for j in range(WIN_NPB):
        xT[j * WIN_K: j * WIN_K + NUM_VARS] = x.T
        xT[j * WIN_K + ONES_ROW] = 1.0
        xT[j * WIN_K + GRID_W_ROW] = np.tile(grid, B // G)
        xT[j * WIN_K + GRID_1_ROW] = 1.0

    in_maps = []
    for core in range(NCORES):
        m = dict(w)
        xc = x[core * BC: (core + 1) * BC]  # [1024, 8]
        m["xT"] = np.ascontiguousarray(xT[:, core * BC: (core + 1) * BC])

        t = (xc - GLO) / GSTEP
        kbase = np.clip(np.floor(t).astype(np.int64), 1, G - 3)
        f = (t - kbase).astype(np.float32)
        bk = (
            -0.5 * f ** 3 + f ** 2 - 0.5 * f,
            1.5 * f ** 3 - 2.5 * f ** 2 + 1.0,
            -1.5 * f ** 3 + 2.0 * f ** 2 + 0.5 * f,
            0.5 * f ** 3 - 0.5 * f ** 2,
        )
        basis = np.zeros((HID, N1 * 3 * HID), np.float32)
        idx = np.zeros((HID, 4 * N1 * 8), np.int16)
        smp = np.arange(BC)
        gq, ii = smp // HID, smp % HID
        slot_of = {0: 0, 2: 1, 3: 2}
        for k in range(4):
            for j in range(N1):
                kv = (kbase[:, j] + (k - 1)).astype(np.int16)
                idx[16 * gq + ii % 16, (k * N1 + j) * 8 + ii // 16] = kv
                if k == 1:
                    continue
                slot = slot_of[k]
                for r in range(16):
                    basis[16 * gq + r, (j * 3 + slot) * HID + ii] = bk[k][:, j]
        m["idx"] = idx
        m["basis"] = basis
        in_maps.append(m)
    return in_maps


def kernel(**inputs):
    from concourse.bass_utils import run_bass_kernel_spmd

    if "nc" not in _CACHE:
        _CACHE["nc"] = _build_bass()
    nc = _CACHE["nc"]

    in_maps = make_in_maps(inputs)
    res = run_bass_kernel_spmd(nc, in_maps, core_ids=list(range(NCORES)))
    outs = []
    smp = np.arange(BC)
    for r in res.results:
        o = r["out"].reshape(-1).astype(np.float64)
        y = r["out2"]
        o += y[16 * (smp // HID), smp % HID]
        outs.append(o)
    return np.concatenate(outs).astype(np.float32)[:, None]
